# revision 15
# baseline (speedup 1.0000x reference)
"""CapsuleLayer kernel for 8 Trainium2 NeuronCores.

Math: with b0 = 0, softmax(b0, axis=1) is exactly uniform (1/N), so
outputs[b,i,k] = squash_k((1/N) * sum_j inputs_hat[b,j,k]) independent of i.
The b-update keeps b constant along axis 1, so softmax stays exactly uniform
and all routing iterations return the same outputs. Hence:

    Wsum[m,k] = sum_j W[j,m,k]
    v[b,k]    = (1/N) * (inputs @ Wsum)[b,k]
    out[b,i,k] = squash_k(v)[b,k]          (broadcast over i)

Kernel 1 (m-sharded): core c reduces W[:, 32c:32c+32, :] over j -> Wsum rows.
Kernel 2 (batch-sharded): core c computes squash((inputs_c @ Wsum)/N) and
broadcast-writes its [64, 256, 256] output slice.
"""

import numpy as np

import concourse.bass as bass
import concourse.mybir as mybir
import concourse.tile as tile
from concourse.ap import AP
from concourse.bass_utils import run_bass_kernel_spmd

F32 = mybir.dt.float32

B, N = 512, 256
NCORES = 8
BPC = B // NCORES  # 64 batch rows per core (kernel 2)
MPC = N // NCORES  # 32 m rows per core (kernel 1)
REPS = 64          # output i-rows written per partition per output DMA
EPS = 1e-7

_CACHE = {}


def _fix_multiwait(nc, maxw=1):
    """This walrus build rejects instructions carrying more than one sync
    wait ("Too many sync wait commands"). Hoist extra waits into standalone
    single-wait EventSemaphore instructions on the same engine, placed
    immediately before the offender."""
    ctr = 0
    for fn in nc.m.functions:
        for bb in fn.blocks:
            out = []
            for ins in bb.instructions:
                si = ins.sync_info
                if si is not None and len(si.on_wait) > maxw:
                    waits = list(si.on_wait)
                    for w in waits[:-maxw]:
                        ctr += 1
                        ev = mybir.InstEventSemaphore(
                            name=f"mwsplit-{ctr}",
                            engine=ins.engine,
                            ins=[],
                            outs=[],
                            sync_info=mybir.SyncInfo(on_wait=[w], on_update=[]),
                        )
                        nc.register_instruction(ev, overwrite=True)
                        out.append(ev)
                    si.on_wait = waits[-maxw:]
                    ins.sync_info = si
                out.append(ins)
            bb.instructions[:] = out
    return nc

# Exec times (ns) of the last traced run, for test harnesses.
LAST_EXEC_NS = {"k1": None, "k2": None}


def _build_k1():
    """Reduce the per-core W slice over j.

    Input  w_in [256 (j), 8192 (m_local*256 + k)]  (= W[:, mslice, :] flat)
    Output wsum_part [1, 8192]  (= Wsum[mslice, :] flat)

    Pipeline per chunk: DMA both j-halves, DVE-add them (j 256->128),
    then PE ones-matmuls reduce the 128 partitions; DVE copies PSUM->acc.
    Chunks alternate between the two HWDGE rings (sync/scalar) so each
    ring's per-DMA setup gap hides behind the other ring's transfer, and
    the add/copy work alternates vector/gpsimd so neither engine lags the
    DMA stream. The _fix_multiwait post-pass legalizes any multi-wait
    instruction, so loads/compute overlap freely.
    """
    nc = bass.Bass()
    FREE = MPC * N    # 8192
    MMF = 512         # moving free dim per matmul
    F32R = mybir.dt.float32r

    # w_in is declared float32r (same bits as fp32 at the runtime boundary;
    # mybir.dt.np(float32r) is np.float32) so the whole load path is typed
    # f32r and the BIR verifier accepts the f32r matmuls.
    w = nc.dram_tensor("w_in", [N, FREE], F32R, kind="ExternalInput")
    wsum = nc.dram_tensor("wsum_part", [1, FREE], F32, kind="ExternalOutput")

    # Both rings together sustain ~400 GB/s, ~200 each; chunks alternate
    # rings, small chunks first so PE starts ~11 us in, small last chunks
    # keep the post-load tail to one cheap matmul group.
    CHUNKS = [512, 512, 2048, 2048, 1024, 1024, 512, 512]
    assert sum(CHUNKS) == FREE

    with tile.TileContext(nc) as tc:
        with (
            tc.tile_pool(name="singles", bufs=1) as singles,
            tc.tile_pool(name="psum", bufs=8, space="PSUM") as psum_pool,
        ):
            ones32 = singles.tile([128, 1], F32)
            nc.vector.memset(ones32[:], 1.0)
            ones = singles.tile([128, 1], F32R)
            nc.vector.tensor_copy(out=ones[:], in_=ones32[:])
            acc = singles.tile([1, FREE], F32)

            # Issue every load up front so no DMA-issuing engine queue ever
            # stalls behind a semaphore-waiting compute instruction.
            tiles = []
            off = 0
            for ci, chunk in enumerate(CHUNKS):
                sl = slice(off, off + chunk)
                ldeng = nc.sync if ci % 2 == 0 else nc.scalar
                ta = singles.tile([128, chunk], F32R, tag=f"ta{ci}")
                ldeng.dma_start(out=ta[:], in_=w[0:128, sl])
                tb = singles.tile([128, chunk], F32R, tag=f"tb{ci}")
                ldeng.dma_start(out=tb[:], in_=w[128:256, sl])
                tiles.append((off, chunk, ta, tb))
                off += chunk

            # PE reduces both raw fp32 j-halves directly as float32r (1
            # col/cycle at moving dim >= 256, vs fp32's 4): two accumulating
            # ones-matmuls per 512-col group, no DVE pre-add at all. The
            # only vector work is the 16 cheap [1,512] PSUM->acc copies.
            for ci, (off, chunk, ta, tb) in enumerate(tiles):
                for g0 in range(0, chunk, MMF):
                    gw = min(MMF, chunk - g0)
                    ps = psum_pool.tile([1, gw], F32)
                    nc.tensor.matmul(
                        ps[:], lhsT=ones[:], rhs=ta[:, g0:g0 + gw],
                        start=True, stop=False,
                    )
                    nc.tensor.matmul(
                        ps[:], lhsT=ones[:], rhs=tb[:, g0:g0 + gw],
                        start=False, stop=True,
                    )
                    nc.vector.tensor_copy(
                        out=acc[0:1, off + g0:off + g0 + gw], in_=ps[:]
                    )

            nc.sync.dma_start(out=wsum[:], in_=acc[:])
    return nc


def _build_k2(REPS_=REPS):
    """Per-core: u = inputs_c @ Wsum, s = squash(u/N), broadcast-write output.

    Inputs  xt   [256 (m), 64 (b)]   (= inputs_c.T)
            wsum [256 (m), 256 (k)]
    Output  out  [BPC*N*N] flat = out[b, i, k] with value s[b, k].

    PSUM partition q = 2*b + ihalf (interleaved duplicate of b), so the flat
    output address q*(N*128) + g*(REPS_*N) + t is affine per DMA g.

    The squash row is physically replicated RPHY times in SBUF so each
    output descriptor covers RPHY*N*4 bytes; 1 KB descriptors cap a HWDGE
    ring at ~160 GB/s (descriptor-generation bound), 4 KB descriptors lift
    that above the HBM limit. The matmul-gating wsum loads go on the sync
    ring and xt on the scalar ring, which also warms both rings so the
    output DMAs skip the ~5 us cold-ring ramp.
    """
    nc = bass.Bass()
    xt = nc.dram_tensor("xt", [N, BPC], F32, kind="ExternalInput")
    ws = nc.dram_tensor("wsum", [N, N], F32, kind="ExternalInput")
    out = nc.dram_tensor("out", [BPC * N * N], F32, kind="ExternalOutput")

    RPHY = 4                    # physical replicas of the squash row in SBUF
    SREP_W = REPS_ * N          # output elements per partition per DMA
    NDMA = (N // 2) // REPS_    # output DMAs, one per group of REPS_ i-rows

    with tile.TileContext(nc) as tc:
        with (
            tc.tile_pool(name="sb", bufs=1) as sb,
            tc.tile_pool(name="psum", bufs=1, space="PSUM") as psum_pool,
        ):
            # Matmul-gating Wsum halves on the sync ring; xt halves on the
            # scalar ring (contraction dim m on partitions for both).
            ws0 = sb.tile([128, N], F32)
            nc.sync.dma_start(out=ws0[:], in_=ws[0:128, :])
            ws1 = sb.tile([128, N], F32)
            nc.sync.dma_start(out=ws1[:], in_=ws[128:256, :])
            xt0 = sb.tile([128, BPC], F32)
            nc.scalar.dma_start(out=xt0[:], in_=xt[0:128, :])
            xt1 = sb.tile([128, BPC], F32)
            nc.scalar.dma_start(out=xt1[:], in_=xt[128:256, :])

            # Duplicate b columns interleaved: xd[:, 2b + d] = xt[:, b].
            # (A stride-0 lhsT AP would avoid the copies, but the BIR
            # verifier requires the stationary operand to have exactly one
            # free dimension.) On gpsimd so the vector engine is free.
            xd0 = sb.tile([128, 2 * BPC], F32)
            xd1 = sb.tile([128, 2 * BPC], F32)
            for xd, xsrc in ((xd0, xt0), (xd1, xt1)):
                pairs = xd[:].rearrange("p (b two) -> p b two", two=2)
                nc.gpsimd.tensor_copy(out=pairs[:, :, 0], in_=xsrc[:])
                nc.gpsimd.tensor_copy(out=pairs[:, :, 1], in_=xsrc[:])

            # u[q, k] = sum_m inputs_c[q//2, m] * Wsum[m, k]
            u = psum_pool.tile([128, N], F32)
            nc.tensor.matmul(u[:], lhsT=xd0[:], rhs=ws0[:], start=True, stop=False)
            nc.tensor.matmul(u[:], lhsT=xd1[:], rhs=ws1[:], start=False, stop=True)

            # squash: v = u/N; s2 = sum_k v^2; s = v * s2/(1+s2)/sqrt(s2+eps)
            #       = u * factor,  factor = s2/(1+s2)/sqrt(s2+eps)/N
            # (eps dropped: s2 ~ 256 here, so it shifts sqrt by ~2e-10 rel.)
            sq = sb.tile([128, N], F32)
            s2 = sb.tile([128, 1], F32)
            nc.scalar.activation(
                out=sq[:], in_=u[:], func=mybir.ActivationFunctionType.Square,
                scale=1.0 / N, accum_out=s2[:],
            )
            r = sb.tile([128, 1], F32)
            nc.scalar.activation(
                out=r[:], in_=s2[:], func=mybir.ActivationFunctionType.Sqrt,
            )
            den = sb.tile([128, 1], F32)
            nc.vector.scalar_tensor_tensor(
                den[:], s2[:], 1.0, r[:],
                op0=mybir.AluOpType.add, op1=mybir.AluOpType.mult,
            )
            rec = sb.tile([128, 1], F32)
            nc.vector.reciprocal(rec[:], den[:])
            fac = sb.tile([128, 1], F32)
            nc.vector.scalar_tensor_tensor(
                fac[:], s2[:], 1.0 / N, rec[:],
                op0=mybir.AluOpType.mult, op1=mybir.AluOpType.mult,
            )

            # srep[q, rep*N + k] = s[q//2, k] for rep < RPHY; the four
            # replicas are computed straight from PSUM u, two on the Act
            # engine and two on DVE, so they run in parallel after fac.
            srep = sb.tile([128, RPHY * N], F32)
            for rep in range(RPHY):
                dst_sl = srep[:, rep * N:(rep + 1) * N]
                if rep % 2 == 0:
                    nc.vector.tensor_scalar(
                        dst_sl, u[:], fac[:], None, mybir.AluOpType.mult
                    )
                else:
                    nc.scalar.activation(
                        out=dst_sl, in_=u[:],
                        func=mybir.ActivationFunctionType.Copy, scale=fac[:],
                    )

            # DMA g writes out[q*32768 + g*SREP_W + grp*RPHY*N + t] =
            # srep[q, t], i.e. b = q//2, i = (q%2)*128 + g*REPS_ + grp*RPHY
            # + t//N, k = t%N. Descriptors are RPHY*N*4 = 4 KB.
            src = AP(
                tensor=srep.tensor,
                offset=srep[:].offset,
                ap=[srep[:].ap[0], [0, REPS_ // RPHY], [1, RPHY * N]],
            )
            for g in range(NDMA):
                dst = AP(
                    tensor=out,
                    offset=g * SREP_W,
                    ap=[[128 * N, 128], [RPHY * N, REPS_ // RPHY], [1, RPHY * N]],
                )
                eng = nc.sync if g % 2 == 0 else nc.scalar
                eng.dma_start(out=dst, in_=src)
    return nc


def _build_fused(REPS_=REPS):
    """Single-launch kernel: fp32r W-reduction, on-device AllGather of the
    per-core Wsum slice, then squash + broadcast output write.

    Inputs  w_in [256 (j), 8192 (m_local*256+k)]  (= W[:, mslice_c, :] flat,
            float32r view of the fp32 bits)
            xt   [256 (m), 64 (b)]                (= inputs_c.T)
    Output  out  [BPC*N*N] flat = out[b, i, k] with value squash(u/N)[b, k].

    Saves the second launch's ~11 us fixed cost if the 32 KB-per-core
    AllGather is cheap enough.
    """
    nc = bass.Bass(num_devices=NCORES)
    FREE = MPC * N
    MMF = 512
    F32R = mybir.dt.float32r
    RPHY = 4
    SREP_W = REPS_ * N
    NDMA = (N // 2) // REPS_

    w = nc.dram_tensor("w_in", [N, FREE], F32R, kind="ExternalInput")
    xt = nc.dram_tensor("xt", [N, BPC], F32, kind="ExternalInput")
    out = nc.dram_tensor("out", [BPC * N * N], F32, kind="ExternalOutput")

    CHUNKS = [512, 512, 2048, 2048, 1024, 1024, 512, 512]
    assert sum(CHUNKS) == FREE

    with tile.TileContext(nc) as tc:
        with (
            tc.tile_pool(name="sb", bufs=1) as sb,
            tc.tile_pool(name="psА", bufs=7, space="PSUM") as psum_a,
            tc.tile_pool(name="psB", bufs=1, space="PSUM") as psum_b,
            tc.tile_pool(name="dram", bufs=1, space="DRAM") as dram,
        ):
            ones32 = sb.tile([128, 1], F32)
            nc.vector.memset(ones32[:], 1.0)
            ones = sb.tile([128, 1], F32R)
            nc.vector.tensor_copy(out=ones[:], in_=ones32[:])
            acc = sb.tile([1, FREE], F32)

            # xt halves early on the scalar ring; W chunks on both rings.
            xt0 = sb.tile([128, BPC], F32)
            nc.scalar.dma_start(out=xt0[:], in_=xt[0:128, :])
            xt1 = sb.tile([128, BPC], F32)
            nc.scalar.dma_start(out=xt1[:], in_=xt[128:256, :])

            tiles = []
            off = 0
            for ci, chunk in enumerate(CHUNKS):
                sl = slice(off, off + chunk)
                ldeng = nc.sync if ci % 2 == 0 else nc.scalar
                ta = sb.tile([128, chunk], F32R, tag=f"ta{ci}")
                ldeng.dma_start(out=ta[:], in_=w[0:128, sl])
                tb = sb.tile([128, chunk], F32R, tag=f"tb{ci}")
                ldeng.dma_start(out=tb[:], in_=w[128:256, sl])
                tiles.append((off, chunk, ta, tb))
                off += chunk

            # b-duplication for the phase-B matmul, on gpsimd during loads.
            xd0 = sb.tile([128, 2 * BPC], F32)
            xd1 = sb.tile([128, 2 * BPC], F32)
            for xd, xsrc in ((xd0, xt0), (xd1, xt1)):
                pairs = xd[:].rearrange("p (b two) -> p b two", two=2)
                nc.gpsimd.tensor_copy(out=pairs[:, :, 0], in_=xsrc[:])
                nc.gpsimd.tensor_copy(out=pairs[:, :, 1], in_=xsrc[:])

            # Phase A: fp32r ones-matmul reduction of both j-halves.
            for ci, (coff, chunk, ta, tb) in enumerate(tiles):
                for g0 in range(0, chunk, MMF):
                    gw = min(MMF, chunk - g0)
                    ps = psum_a.tile([1, gw], F32)
                    nc.tensor.matmul(
                        ps[:], lhsT=ones[:], rhs=ta[:, g0:g0 + gw],
                        start=True, stop=False,
                    )
                    nc.tensor.matmul(
                        ps[:], lhsT=ones[:], rhs=tb[:, g0:g0 + gw],
                        start=False, stop=True,
                    )
                    nc.vector.tensor_copy(
                        out=acc[0:1, coff + g0:coff + g0 + gw], in_=ps[:]
                    )

            # Exchange: acc -> DRAM bounce -> AllGather -> Wsum [256, 256].
            ag_in = dram.tile([1, FREE], F32)
            ag_out = dram.tile([NCORES, FREE], F32)
            nc.gpsimd.dma_start(ag_in[:], acc[:])
            nc.gpsimd.collective_compute(
                "AllGather",
                mybir.AluOpType.bypass,
                replica_groups=[list(range(NCORES))],
                ins=[ag_in[:].opt()],
                outs=[ag_out[:].opt()],
            )
            # ag_out flat [8, 32*256] is exactly Wsum [256 (m), 256 (k)].
            ws0 = sb.tile([128, N], F32)
            ws1 = sb.tile([128, N], F32)
            base = ag_out[:]
            nc.sync.dma_start(
                out=ws0[:],
                in_=AP(tensor=base.tensor, offset=base.offset,
                       ap=[[N, 128], [1, N]]),
            )
            nc.scalar.dma_start(
                out=ws1[:],
                in_=AP(tensor=base.tensor, offset=base.offset + 128 * N,
                       ap=[[N, 128], [1, N]]),
            )

            # Phase B: u = inputs_c @ Wsum (PSUM q = 2b + ihalf), squash,
            # replicate, write. Identical to _build_k2's tail.
            u = psum_b.tile([128, N], F32)
            nc.tensor.matmul(u[:], lhsT=xd0[:], rhs=ws0[:], start=True, stop=False)
            nc.tensor.matmul(u[:], lhsT=xd1[:], rhs=ws1[:], start=False, stop=True)

            sq = sb.tile([128, N], F32)
            s2 = sb.tile([128, 1], F32)
            nc.scalar.activation(
                out=sq[:], in_=u[:], func=mybir.ActivationFunctionType.Square,
                scale=1.0 / N, accum_out=s2[:],
            )
            r = sb.tile([128, 1], F32)
            nc.scalar.activation(
                out=r[:], in_=s2[:], func=mybir.ActivationFunctionType.Sqrt,
            )
            den = sb.tile([128, 1], F32)
            nc.vector.scalar_tensor_tensor(
                den[:], s2[:], 1.0, r[:],
                op0=mybir.AluOpType.add, op1=mybir.AluOpType.mult,
            )
            rec = sb.tile([128, 1], F32)
            nc.vector.reciprocal(rec[:], den[:])
            fac = sb.tile([128, 1], F32)
            nc.vector.scalar_tensor_tensor(
                fac[:], s2[:], 1.0 / N, rec[:],
                op0=mybir.AluOpType.mult, op1=mybir.AluOpType.mult,
            )

            srep = sb.tile([128, RPHY * N], F32)
            for rep in range(RPHY):
                dst_sl = srep[:, rep * N:(rep + 1) * N]
                if rep % 2 == 0:
                    nc.vector.tensor_scalar(
                        dst_sl, u[:], fac[:], None, mybir.AluOpType.mult
                    )
                else:
                    nc.scalar.activation(
                        out=dst_sl, in_=u[:],
                        func=mybir.ActivationFunctionType.Copy, scale=fac[:],
                    )

            src = AP(
                tensor=srep.tensor,
                offset=srep[:].offset,
                ap=[srep[:].ap[0], [0, REPS_ // RPHY], [1, RPHY * N]],
            )
            for g in range(NDMA):
                dst = AP(
                    tensor=out,
                    offset=g * SREP_W,
                    ap=[[128 * N, 128], [RPHY * N, REPS_ // RPHY], [1, RPHY * N]],
                )
                eng = nc.sync if g % 2 == 0 else nc.scalar
                eng.dma_start(out=dst, in_=src)
    return nc


def _run(nc, in_maps, core_ids, trace):
    if trace:
        try:
            return run_bass_kernel_spmd(nc, in_maps, core_ids, trace=True)
        except Exception as e:  # noqa: BLE001
            print(f"kernel: trace run failed ({e}); rerunning without trace")
    return run_bass_kernel_spmd(nc, in_maps, core_ids, trace=False)


def _get(name):
    if name not in _CACHE:
        builders = {"k1": _build_k1, "k2": _build_k2, "fused": _build_fused}
        _CACHE[name] = _fix_multiwait(builders[name]())
    return _CACHE[name]


USE_FUSED = True


def kernel(inputs: np.ndarray, W: np.ndarray, trace: bool = False) -> np.ndarray:
    inputs = np.ascontiguousarray(inputs, dtype=np.float32)
    W = np.ascontiguousarray(W, dtype=np.float32)
    core_ids = list(range(NCORES))

    if USE_FUSED:
        try:
            return _kernel_fused(inputs, W, core_ids, trace)
        except Exception as e:  # noqa: BLE001
            print(f"kernel: fused path failed ({e}); falling back to 2-launch")

    return _kernel_split(inputs, W, core_ids, trace)


def _kernel_fused(inputs, W, core_ids, trace):
    fused = _get("fused")
    xt_full = np.ascontiguousarray(inputs.T)  # [256, 512]
    in_maps = [
        {
            "w_in": np.ascontiguousarray(
                W[:, c * MPC:(c + 1) * MPC, :]
            ).reshape(N, MPC * N),
            "xt": np.ascontiguousarray(xt_full[:, c * BPC:(c + 1) * BPC]),
        }
        for c in core_ids
    ]
    res = _run(fused, in_maps, core_ids, trace)
    LAST_EXEC_NS["k1"] = res.exec_time_ns
    LAST_EXEC_NS["k2"] = 0
    return np.concatenate(
        [res.results[c]["out"].reshape(BPC, N, N) for c in core_ids], axis=0
    )


def _kernel_split(inputs, W, core_ids, trace):
    # ---- kernel 1: Wsum rows, m-sharded ----
    k1 = _get("k1")
    in_maps1 = [
        {
            "w_in": np.ascontiguousarray(
                W[:, c * MPC:(c + 1) * MPC, :]
            ).reshape(N, MPC * N)
        }
        for c in core_ids
    ]
    res1 = _run(k1, in_maps1, core_ids, trace)
    LAST_EXEC_NS["k1"] = res1.exec_time_ns
    wsum = np.concatenate(
        [res1.results[c]["wsum_part"].reshape(MPC, N) for c in core_ids], axis=0
    )  # [256, 256]

    # ---- kernel 2: squash + broadcast write, batch-sharded ----
    k2 = _get("k2")
    xt_full = np.ascontiguousarray(inputs.T)  # [256, 512]
    in_maps2 = [
        {
            "xt": np.ascontiguousarray(xt_full[:, c * BPC:(c + 1) * BPC]),
            "wsum": wsum,
        }
        for c in core_ids
    ]
    res2 = _run(k2, in_maps2, core_ids, trace)
    LAST_EXEC_NS["k2"] = res2.exec_time_ns
    out = np.concatenate(
        [res2.results[c]["out"].reshape(BPC, N, N) for c in core_ids], axis=0
    )
    return out



# revision 16
# speedup vs baseline: 1.4584x; 1.4584x over previous
"""CapsuleLayer kernel for 8 Trainium2 NeuronCores.

Math: with b0 = 0, softmax(b0, axis=1) is exactly uniform (1/N), so
outputs[b,i,k] = squash_k((1/N) * sum_j inputs_hat[b,j,k]) independent of i.
The b-update keeps b constant along axis 1, so softmax stays exactly uniform
and all routing iterations return the same outputs. Hence:

    Wsum[m,k] = sum_j W[j,m,k]
    v[b,k]    = (1/N) * (inputs @ Wsum)[b,k]
    out[b,i,k] = squash_k(v)[b,k]          (broadcast over i)

Kernel 1 (m-sharded): core c reduces W[:, 32c:32c+32, :] over j -> Wsum rows.
Kernel 2 (batch-sharded): core c computes squash((inputs_c @ Wsum)/N) and
broadcast-writes its [64, 256, 256] output slice.
"""

import numpy as np

import concourse.bass as bass
import concourse.mybir as mybir
import concourse.tile as tile
from concourse.ap import AP
from concourse.bass_utils import run_bass_kernel_spmd

F32 = mybir.dt.float32

B, N = 512, 256
NCORES = 8
BPC = B // NCORES  # 64 batch rows per core (kernel 2)
MPC = N // NCORES  # 32 m rows per core (kernel 1)
REPS = 64          # output i-rows written per partition per output DMA
EPS = 1e-7

_CACHE = {}


def _fix_multiwait(nc, maxw=1):
    """This walrus build rejects instructions carrying more than one sync
    wait ("Too many sync wait commands"). Hoist extra waits into standalone
    single-wait EventSemaphore instructions on the same engine, placed
    immediately before the offender."""
    ctr = 0
    for fn in nc.m.functions:
        for bb in fn.blocks:
            out = []
            for ins in bb.instructions:
                si = ins.sync_info
                if si is not None and len(si.on_wait) > maxw:
                    waits = list(si.on_wait)
                    for w in waits[:-maxw]:
                        ctr += 1
                        ev = mybir.InstEventSemaphore(
                            name=f"mwsplit-{ctr}",
                            engine=ins.engine,
                            ins=[],
                            outs=[],
                            sync_info=mybir.SyncInfo(on_wait=[w], on_update=[]),
                        )
                        nc.register_instruction(ev, overwrite=True)
                        out.append(ev)
                    si.on_wait = waits[-maxw:]
                    ins.sync_info = si
                out.append(ins)
            bb.instructions[:] = out
    return nc

# Exec times (ns) of the last traced run, for test harnesses.
LAST_EXEC_NS = {"k1": None, "k2": None}


def _build_k1():
    """Reduce the per-core W slice over j.

    Input  w_in [256 (j), 8192 (m_local*256 + k)]  (= W[:, mslice, :] flat)
    Output wsum_part [1, 8192]  (= Wsum[mslice, :] flat)

    Pipeline per chunk: DMA both j-halves, DVE-add them (j 256->128),
    then PE ones-matmuls reduce the 128 partitions; DVE copies PSUM->acc.
    Chunks alternate between the two HWDGE rings (sync/scalar) so each
    ring's per-DMA setup gap hides behind the other ring's transfer, and
    the add/copy work alternates vector/gpsimd so neither engine lags the
    DMA stream. The _fix_multiwait post-pass legalizes any multi-wait
    instruction, so loads/compute overlap freely.
    """
    nc = bass.Bass()
    FREE = MPC * N    # 8192
    MMF = 512         # moving free dim per matmul
    F32R = mybir.dt.float32r

    # w_in is declared float32r (same bits as fp32 at the runtime boundary;
    # mybir.dt.np(float32r) is np.float32) so the whole load path is typed
    # f32r and the BIR verifier accepts the f32r matmuls.
    w = nc.dram_tensor("w_in", [N, FREE], F32R, kind="ExternalInput")
    wsum = nc.dram_tensor("wsum_part", [1, FREE], F32, kind="ExternalOutput")

    # Both rings together sustain ~400 GB/s, ~200 each; chunks alternate
    # rings, small chunks first so PE starts ~11 us in, small last chunks
    # keep the post-load tail to one cheap matmul group.
    CHUNKS = [512, 512, 2048, 2048, 1024, 1024, 512, 512]
    assert sum(CHUNKS) == FREE

    with tile.TileContext(nc) as tc:
        with (
            tc.tile_pool(name="singles", bufs=1) as singles,
            tc.tile_pool(name="psum", bufs=8, space="PSUM") as psum_pool,
        ):
            ones32 = singles.tile([128, 1], F32)
            nc.vector.memset(ones32[:], 1.0)
            ones = singles.tile([128, 1], F32R)
            nc.vector.tensor_copy(out=ones[:], in_=ones32[:])
            acc = singles.tile([1, FREE], F32)

            # Issue every load up front so no DMA-issuing engine queue ever
            # stalls behind a semaphore-waiting compute instruction.
            tiles = []
            off = 0
            for ci, chunk in enumerate(CHUNKS):
                sl = slice(off, off + chunk)
                ldeng = nc.sync if ci % 2 == 0 else nc.scalar
                ta = singles.tile([128, chunk], F32R, tag=f"ta{ci}")
                ldeng.dma_start(out=ta[:], in_=w[0:128, sl])
                tb = singles.tile([128, chunk], F32R, tag=f"tb{ci}")
                ldeng.dma_start(out=tb[:], in_=w[128:256, sl])
                tiles.append((off, chunk, ta, tb))
                off += chunk

            # PE reduces both raw fp32 j-halves directly as float32r (1
            # col/cycle at moving dim >= 256, vs fp32's 4): two accumulating
            # ones-matmuls per 512-col group, no DVE pre-add at all. The
            # only vector work is the 16 cheap [1,512] PSUM->acc copies.
            for ci, (off, chunk, ta, tb) in enumerate(tiles):
                for g0 in range(0, chunk, MMF):
                    gw = min(MMF, chunk - g0)
                    ps = psum_pool.tile([1, gw], F32)
                    nc.tensor.matmul(
                        ps[:], lhsT=ones[:], rhs=ta[:, g0:g0 + gw],
                        start=True, stop=False,
                    )
                    nc.tensor.matmul(
                        ps[:], lhsT=ones[:], rhs=tb[:, g0:g0 + gw],
                        start=False, stop=True,
                    )
                    nc.vector.tensor_copy(
                        out=acc[0:1, off + g0:off + g0 + gw], in_=ps[:]
                    )

            nc.sync.dma_start(out=wsum[:], in_=acc[:])
    return nc


def _build_k2(REPS_=REPS):
    """Per-core: u = inputs_c @ Wsum, s = squash(u/N), broadcast-write output.

    Inputs  xt   [256 (m), 64 (b)]   (= inputs_c.T)
            wsum [256 (m), 256 (k)]
    Output  out  [BPC*N*N] flat = out[b, i, k] with value s[b, k].

    PSUM partition q = 2*b + ihalf (interleaved duplicate of b), so the flat
    output address q*(N*128) + g*(REPS_*N) + t is affine per DMA g.

    The squash row is physically replicated RPHY times in SBUF so each
    output descriptor covers RPHY*N*4 bytes; 1 KB descriptors cap a HWDGE
    ring at ~160 GB/s (descriptor-generation bound), 4 KB descriptors lift
    that above the HBM limit. The matmul-gating wsum loads go on the sync
    ring and xt on the scalar ring, which also warms both rings so the
    output DMAs skip the ~5 us cold-ring ramp.
    """
    nc = bass.Bass()
    xt = nc.dram_tensor("xt", [N, BPC], F32, kind="ExternalInput")
    ws = nc.dram_tensor("wsum", [N, N], F32, kind="ExternalInput")
    out = nc.dram_tensor("out", [BPC * N * N], F32, kind="ExternalOutput")

    RPHY = 4                    # physical replicas of the squash row in SBUF
    SREP_W = REPS_ * N          # output elements per partition per DMA
    NDMA = (N // 2) // REPS_    # output DMAs, one per group of REPS_ i-rows

    with tile.TileContext(nc) as tc:
        with (
            tc.tile_pool(name="sb", bufs=1) as sb,
            tc.tile_pool(name="psum", bufs=1, space="PSUM") as psum_pool,
        ):
            # Matmul-gating Wsum halves on the sync ring; xt halves on the
            # scalar ring (contraction dim m on partitions for both).
            ws0 = sb.tile([128, N], F32)
            nc.sync.dma_start(out=ws0[:], in_=ws[0:128, :])
            ws1 = sb.tile([128, N], F32)
            nc.sync.dma_start(out=ws1[:], in_=ws[128:256, :])
            xt0 = sb.tile([128, BPC], F32)
            nc.scalar.dma_start(out=xt0[:], in_=xt[0:128, :])
            xt1 = sb.tile([128, BPC], F32)
            nc.scalar.dma_start(out=xt1[:], in_=xt[128:256, :])

            # Duplicate b columns interleaved: xd[:, 2b + d] = xt[:, b].
            # (A stride-0 lhsT AP would avoid the copies, but the BIR
            # verifier requires the stationary operand to have exactly one
            # free dimension.) On gpsimd so the vector engine is free.
            xd0 = sb.tile([128, 2 * BPC], F32)
            xd1 = sb.tile([128, 2 * BPC], F32)
            for xd, xsrc in ((xd0, xt0), (xd1, xt1)):
                pairs = xd[:].rearrange("p (b two) -> p b two", two=2)
                nc.gpsimd.tensor_copy(out=pairs[:, :, 0], in_=xsrc[:])
                nc.gpsimd.tensor_copy(out=pairs[:, :, 1], in_=xsrc[:])

            # u[q, k] = sum_m inputs_c[q//2, m] * Wsum[m, k]
            u = psum_pool.tile([128, N], F32)
            nc.tensor.matmul(u[:], lhsT=xd0[:], rhs=ws0[:], start=True, stop=False)
            nc.tensor.matmul(u[:], lhsT=xd1[:], rhs=ws1[:], start=False, stop=True)

            # squash: v = u/N; s2 = sum_k v^2; s = v * s2/(1+s2)/sqrt(s2+eps)
            #       = u * factor,  factor = s2/(1+s2)/sqrt(s2+eps)/N
            # (eps dropped: s2 ~ 256 here, so it shifts sqrt by ~2e-10 rel.)
            sq = sb.tile([128, N], F32)
            s2 = sb.tile([128, 1], F32)
            nc.scalar.activation(
                out=sq[:], in_=u[:], func=mybir.ActivationFunctionType.Square,
                scale=1.0 / N, accum_out=s2[:],
            )
            r = sb.tile([128, 1], F32)
            nc.scalar.activation(
                out=r[:], in_=s2[:], func=mybir.ActivationFunctionType.Sqrt,
            )
            den = sb.tile([128, 1], F32)
            nc.vector.scalar_tensor_tensor(
                den[:], s2[:], 1.0, r[:],
                op0=mybir.AluOpType.add, op1=mybir.AluOpType.mult,
            )
            rec = sb.tile([128, 1], F32)
            nc.vector.reciprocal(rec[:], den[:])
            fac = sb.tile([128, 1], F32)
            nc.vector.scalar_tensor_tensor(
                fac[:], s2[:], 1.0 / N, rec[:],
                op0=mybir.AluOpType.mult, op1=mybir.AluOpType.mult,
            )

            # srep[q, rep*N + k] = s[q//2, k] for rep < RPHY; the four
            # replicas are computed straight from PSUM u, two on the Act
            # engine and two on DVE, so they run in parallel after fac.
            srep = sb.tile([128, RPHY * N], F32)
            for rep in range(RPHY):
                dst_sl = srep[:, rep * N:(rep + 1) * N]
                if rep % 2 == 0:
                    nc.vector.tensor_scalar(
                        dst_sl, u[:], fac[:], None, mybir.AluOpType.mult
                    )
                else:
                    nc.scalar.activation(
                        out=dst_sl, in_=u[:],
                        func=mybir.ActivationFunctionType.Copy, scale=fac[:],
                    )

            # DMA g writes out[q*32768 + g*SREP_W + grp*RPHY*N + t] =
            # srep[q, t], i.e. b = q//2, i = (q%2)*128 + g*REPS_ + grp*RPHY
            # + t//N, k = t%N. Descriptors are RPHY*N*4 = 4 KB.
            src = AP(
                tensor=srep.tensor,
                offset=srep[:].offset,
                ap=[srep[:].ap[0], [0, REPS_ // RPHY], [1, RPHY * N]],
            )
            for g in range(NDMA):
                dst = AP(
                    tensor=out,
                    offset=g * SREP_W,
                    ap=[[128 * N, 128], [RPHY * N, REPS_ // RPHY], [1, RPHY * N]],
                )
                eng = nc.sync if g % 2 == 0 else nc.scalar
                eng.dma_start(out=dst, in_=src)
    return nc


def _build_fused(REPS_=REPS):
    """Single-launch kernel: fp32r W-reduction, on-device AllGather of the
    per-core Wsum slice, then squash + broadcast output write.

    Inputs  w_in [256 (j), 8192 (m_local*256+k)]  (= W[:, mslice_c, :] flat,
            float32r view of the fp32 bits)
            xt   [256 (m), 64 (b)]                (= inputs_c.T)
    Output  out  [BPC*N*N] flat = out[b, i, k] with value squash(u/N)[b, k].

    Saves the second launch's ~11 us fixed cost if the 32 KB-per-core
    AllGather is cheap enough.
    """
    nc = bass.Bass(num_devices=NCORES)
    FREE = MPC * N
    MMF = 512
    F32R = mybir.dt.float32r
    RPHY = 4
    SREP_W = REPS_ * N
    NDMA = (N // 2) // REPS_

    w = nc.dram_tensor("w_in", [N, FREE], F32R, kind="ExternalInput")
    xt = nc.dram_tensor("xt", [N, BPC], F32, kind="ExternalInput")
    out = nc.dram_tensor("out", [BPC * N * N], F32, kind="ExternalOutput")

    CHUNKS = [512, 512, 2048, 2048, 1024, 1024, 512, 512]
    assert sum(CHUNKS) == FREE

    with tile.TileContext(nc) as tc:
        with (
            tc.tile_pool(name="sb", bufs=1) as sb,
            tc.tile_pool(name="psА", bufs=7, space="PSUM") as psum_a,
            tc.tile_pool(name="psB", bufs=1, space="PSUM") as psum_b,
            tc.tile_pool(name="dram", bufs=1, space="DRAM") as dram,
        ):
            ones32 = sb.tile([128, 1], F32)
            nc.vector.memset(ones32[:], 1.0)
            ones = sb.tile([128, 1], F32R)
            nc.vector.tensor_copy(out=ones[:], in_=ones32[:])
            acc = sb.tile([1, FREE], F32)

            # xt halves early on the scalar ring; W chunks on both rings.
            xt0 = sb.tile([128, BPC], F32)
            nc.scalar.dma_start(out=xt0[:], in_=xt[0:128, :])
            xt1 = sb.tile([128, BPC], F32)
            nc.scalar.dma_start(out=xt1[:], in_=xt[128:256, :])

            tiles = []
            off = 0
            for ci, chunk in enumerate(CHUNKS):
                sl = slice(off, off + chunk)
                ldeng = nc.sync if ci % 2 == 0 else nc.scalar
                ta = sb.tile([128, chunk], F32R, tag=f"ta{ci}")
                ldeng.dma_start(out=ta[:], in_=w[0:128, sl])
                tb = sb.tile([128, chunk], F32R, tag=f"tb{ci}")
                ldeng.dma_start(out=tb[:], in_=w[128:256, sl])
                tiles.append((off, chunk, ta, tb))
                off += chunk

            # b-duplication for the phase-B matmul, on gpsimd during loads.
            xd0 = sb.tile([128, 2 * BPC], F32)
            xd1 = sb.tile([128, 2 * BPC], F32)
            for xd, xsrc in ((xd0, xt0), (xd1, xt1)):
                pairs = xd[:].rearrange("p (b two) -> p b two", two=2)
                nc.gpsimd.tensor_copy(out=pairs[:, :, 0], in_=xsrc[:])
                nc.gpsimd.tensor_copy(out=pairs[:, :, 1], in_=xsrc[:])

            # Phase A: fp32r ones-matmul reduction of both j-halves.
            for ci, (coff, chunk, ta, tb) in enumerate(tiles):
                for g0 in range(0, chunk, MMF):
                    gw = min(MMF, chunk - g0)
                    ps = psum_a.tile([1, gw], F32)
                    nc.tensor.matmul(
                        ps[:], lhsT=ones[:], rhs=ta[:, g0:g0 + gw],
                        start=True, stop=False,
                    )
                    nc.tensor.matmul(
                        ps[:], lhsT=ones[:], rhs=tb[:, g0:g0 + gw],
                        start=False, stop=True,
                    )
                    nc.vector.tensor_copy(
                        out=acc[0:1, coff + g0:coff + g0 + gw], in_=ps[:]
                    )

            # Exchange: acc -> DRAM bounce -> AllGather -> Wsum [256, 256].
            ag_in = dram.tile([1, FREE], F32)
            ag_out = dram.tile([NCORES, FREE], F32)
            nc.gpsimd.dma_start(ag_in[:], acc[:])
            nc.gpsimd.collective_compute(
                "AllGather",
                mybir.AluOpType.bypass,
                replica_groups=[list(range(NCORES))],
                ins=[ag_in[:].opt()],
                outs=[ag_out[:].opt()],
            )
            # ag_out flat [8, 32*256] is exactly Wsum [256 (m), 256 (k)].
            ws0 = sb.tile([128, N], F32)
            ws1 = sb.tile([128, N], F32)
            base = ag_out[:]
            nc.sync.dma_start(
                out=ws0[:],
                in_=AP(tensor=base.tensor, offset=base.offset,
                       ap=[[N, 128], [1, N]]),
            )
            nc.scalar.dma_start(
                out=ws1[:],
                in_=AP(tensor=base.tensor, offset=base.offset + 128 * N,
                       ap=[[N, 128], [1, N]]),
            )

            # Phase B: u = inputs_c @ Wsum (PSUM q = 2b + ihalf), squash,
            # replicate, write. Identical to _build_k2's tail.
            u = psum_b.tile([128, N], F32)
            nc.tensor.matmul(u[:], lhsT=xd0[:], rhs=ws0[:], start=True, stop=False)
            nc.tensor.matmul(u[:], lhsT=xd1[:], rhs=ws1[:], start=False, stop=True)

            sq = sb.tile([128, N], F32)
            s2 = sb.tile([128, 1], F32)
            nc.scalar.activation(
                out=sq[:], in_=u[:], func=mybir.ActivationFunctionType.Square,
                scale=1.0 / N, accum_out=s2[:],
            )
            r = sb.tile([128, 1], F32)
            nc.scalar.activation(
                out=r[:], in_=s2[:], func=mybir.ActivationFunctionType.Sqrt,
            )
            den = sb.tile([128, 1], F32)
            nc.vector.scalar_tensor_tensor(
                den[:], s2[:], 1.0, r[:],
                op0=mybir.AluOpType.add, op1=mybir.AluOpType.mult,
            )
            rec = sb.tile([128, 1], F32)
            nc.vector.reciprocal(rec[:], den[:])
            fac = sb.tile([128, 1], F32)
            nc.vector.scalar_tensor_tensor(
                fac[:], s2[:], 1.0 / N, rec[:],
                op0=mybir.AluOpType.mult, op1=mybir.AluOpType.mult,
            )

            srep = sb.tile([128, RPHY * N], F32)
            for rep in range(RPHY):
                dst_sl = srep[:, rep * N:(rep + 1) * N]
                if rep % 2 == 0:
                    nc.vector.tensor_scalar(
                        dst_sl, u[:], fac[:], None, mybir.AluOpType.mult
                    )
                else:
                    nc.scalar.activation(
                        out=dst_sl, in_=u[:],
                        func=mybir.ActivationFunctionType.Copy, scale=fac[:],
                    )

            src = AP(
                tensor=srep.tensor,
                offset=srep[:].offset,
                ap=[srep[:].ap[0], [0, REPS_ // RPHY], [1, RPHY * N]],
            )
            for g in range(NDMA):
                dst = AP(
                    tensor=out,
                    offset=g * SREP_W,
                    ap=[[128 * N, 128], [RPHY * N, REPS_ // RPHY], [1, RPHY * N]],
                )
                eng = nc.sync if g % 2 == 0 else nc.scalar
                eng.dma_start(out=dst, in_=src)
    return nc


def _run(nc, in_maps, core_ids, trace):
    if trace:
        try:
            return run_bass_kernel_spmd(nc, in_maps, core_ids, trace=True)
        except Exception as e:  # noqa: BLE001
            print(f"kernel: trace run failed ({e}); rerunning without trace")
    return run_bass_kernel_spmd(nc, in_maps, core_ids, trace=False)


def _get(name):
    if name not in _CACHE:
        builders = {"k1": _build_k1, "k2": _build_k2, "fused": _build_fused}
        _CACHE[name] = _fix_multiwait(builders[name]())
    return _CACHE[name]


USE_FUSED = False  # on-device AllGather measured ~50 us here; 2-launch wins


def kernel(inputs: np.ndarray, W: np.ndarray, trace: bool = False) -> np.ndarray:
    inputs = np.ascontiguousarray(inputs, dtype=np.float32)
    W = np.ascontiguousarray(W, dtype=np.float32)
    core_ids = list(range(NCORES))

    if USE_FUSED:
        try:
            return _kernel_fused(inputs, W, core_ids, trace)
        except Exception as e:  # noqa: BLE001
            print(f"kernel: fused path failed ({e}); falling back to 2-launch")

    return _kernel_split(inputs, W, core_ids, trace)


def _kernel_fused(inputs, W, core_ids, trace):
    fused = _get("fused")
    xt_full = np.ascontiguousarray(inputs.T)  # [256, 512]
    in_maps = [
        {
            "w_in": np.ascontiguousarray(
                W[:, c * MPC:(c + 1) * MPC, :]
            ).reshape(N, MPC * N),
            "xt": np.ascontiguousarray(xt_full[:, c * BPC:(c + 1) * BPC]),
        }
        for c in core_ids
    ]
    res = _run(fused, in_maps, core_ids, trace)
    LAST_EXEC_NS["k1"] = res.exec_time_ns
    LAST_EXEC_NS["k2"] = 0
    return np.concatenate(
        [res.results[c]["out"].reshape(BPC, N, N) for c in core_ids], axis=0
    )


def _kernel_split(inputs, W, core_ids, trace):
    # ---- kernel 1: Wsum rows, m-sharded ----
    k1 = _get("k1")
    in_maps1 = [
        {
            "w_in": np.ascontiguousarray(
                W[:, c * MPC:(c + 1) * MPC, :]
            ).reshape(N, MPC * N)
        }
        for c in core_ids
    ]
    res1 = _run(k1, in_maps1, core_ids, trace)
    LAST_EXEC_NS["k1"] = res1.exec_time_ns
    wsum = np.concatenate(
        [res1.results[c]["wsum_part"].reshape(MPC, N) for c in core_ids], axis=0
    )  # [256, 256]

    # ---- kernel 2: squash + broadcast write, batch-sharded ----
    k2 = _get("k2")
    xt_full = np.ascontiguousarray(inputs.T)  # [256, 512]
    in_maps2 = [
        {
            "xt": np.ascontiguousarray(xt_full[:, c * BPC:(c + 1) * BPC]),
            "wsum": wsum,
        }
        for c in core_ids
    ]
    res2 = _run(k2, in_maps2, core_ids, trace)
    LAST_EXEC_NS["k2"] = res2.exec_time_ns
    out = np.concatenate(
        [res2.results[c]["out"].reshape(BPC, N, N) for c in core_ids], axis=0
    )
    return out

